# revision 38
# baseline (speedup 1.0000x reference)
"""Trainium2 Bass kernel for Mistral-style sliding-window GQA attention.

Problem: B=2, T=2048, C=2048, 32 q heads / 8 kv heads, head_dim=64,
sliding causal window 1024, RoPE, fp32.

Sharding (sequence-parallel, no cross-core communication):
  core c in 0..7 handles batch b=c//4 and contiguous 512-row chunk k=c%4.
  Each core computes q for its 512 rows, k/v for its rows plus a 1024-row
  halo (zero-padded before t=0), full attention for its rows over all 32
  heads, and the output projection for its rows.  Host gathers by
  concatenation only.

Schedule (v5 — keeps the PE continuously fed):
  - KV projection per 512-col third: V-only rounds for ci 0..3 (covering
    the previous third's PSUM WAR), then V+K interleaved rounds, then the
    deferred K rounds for ci 1..3.  x tiles die right after their K
    round, so the stream pool needs only 8 buffers; wk/wv stay resident.
  - RoPE uses a host-pre-shuffled sin table: out = ps*cos + shuf(ps*sinp)
    where the rotate-half shuffle folds into the four quarter ADDs — no
    shuffle copies at all.  kv ropes stage kps PSUM->SBUF with a fast ACT
    copy (frees the PSUM banks inside the next third's V-lead); muls run
    on DVE, adds on Pool (split DVE/Pool for the last third).
  - q sweep 0 runs right after kv (its PE work covers the last third's
    rope drain); its wq tiles prefetch during kv from a top-level pool.
    Sweeps run as HALF-sweeps: group 2p only needs qT rows 0:64 (m4 0,1),
    so half A's rope runs mid-phase with ~20us of slack.
  - The xkv third covering the core's own 512 tokens doubles as the
    Q-projection moving operand (resident tiles, no re-DMA).
  - Q sweeps 1..3 interleave with attention as per-ci rounds filling the
    PE bubbles left by the ACT-bound exp chain; each half-sweep has its
    own 2 PSUM banks so WAR never stalls the interleave.
  - Output projection: oc0's k-rounds interleave into the last attention
    phase; wo tiles stream with a rolling 6-round lookahead from phase 2;
    oc1..3 run after ps_att closes, on double-buffered PSUM banks.
  - scores in S^T=[key, query] layout with the 4 q heads of a kv group
    packed in the moving operand (N=512, keeps float32r at 1 cyc/row);
    PV uses V_ext (64 dims + validity column) as stationary so row 64
    accumulates the softmax denominator; no max-subtraction (inputs are
    N(0,1)-scaled, exp stays in fp32 range).
"""

import ml_dtypes
import numpy as np

import concourse.bass as bass
import concourse.mybir as mybir
import concourse.tile as tile
from concourse import bacc
from concourse.bass_utils import run_bass_kernel_spmd

B, T, C = 2, 2048, 2048
NH, NKV, D = 32, 8, 64
REP = NH // NKV
WIN = 1024
CH = 512          # q rows per core
KVR = CH + WIN    # kv rows per core (with halo)
NCORE = 8
DT = mybir.dt.float32
F32R = mybir.dt.float32r
SCALE = 1.0 / np.sqrt(np.float32(D))
ROPE_BASE = 10000.0

FD = T // 128     # 16 contraction tiles of the model dim
NQT = CH // 128   # 4 q tiles per chunk
NKB = KVR // 128  # 12 kv blocks per core
NWB = 9           # kv blocks in the window of one q tile
VW = 65           # v_ext width per kv block (64 dims + validity column)
VP = NKB * VW     # per-head v_ext pitch (780)


def _rope_write(nc, pool, out_ap, ps, cosw, ssinw, n, add_engine,
                swap_engine=None, stage_engine=None):
    """out = ps*cos + rot_half(ps)*sin on a [128, n] 2-head-packed tile.

    ssinw rows carry the rotate-half signs (rows 0-31/64-95 negated) and
    any folded scale; cosw carries the same scale.  out_ap is either one
    [128, n] AP or a list of two [64, n] halves.

    Two modes:
      - swap_engine=None ("psum" mode): 4 quarter multiplies read ps
        directly with shifted partition bases (legal because ps is PSUM),
        plus the full-width cos multiply, all on DVE.
      - swap_engine given: ps is optionally staged to SBUF (stage_engine,
        frees the PSUM bank fast), the rotate-half shuffle materializes
        via 4 quarter copies on swap_engine, then 2 full DVE multiplies.
    The final adds run on add_engine.
    """
    if swap_engine is not None:
        if stage_engine is not None:
            stg = pool.tile([128, n], DT, tag="rope_stg", name="rope_stg")
            scp = (stage_engine.copy if hasattr(stage_engine, "copy")
                   else stage_engine.tensor_copy)
            scp(stg[:], ps[:])
            ps = stg
        sw = pool.tile([128, n], DT, tag="rope_sw", name="rope_sw")
        cp = (swap_engine.copy if hasattr(swap_engine, "copy")
              else swap_engine.tensor_copy)
        cp(sw[0:32, :], ps[32:64, :])
        cp(sw[32:64, :], ps[0:32, :])
        cp(sw[64:96, :], ps[96:128, :])
        cp(sw[96:128, :], ps[64:96, :])
        t2 = pool.tile([128, n], DT, tag="rope_t2", name="rope_t2")
        nc.vector.tensor_mul(t2[:], sw[:], ssinw[:])
    else:
        t2 = pool.tile([128, n], DT, tag="rope_t2", name="rope_t2")
        nc.vector.tensor_mul(t2[0:32, :], ps[32:64, :], ssinw[0:32, :])
        nc.vector.tensor_mul(t2[32:64, :], ps[0:32, :], ssinw[32:64, :])
        nc.vector.tensor_mul(t2[64:96, :], ps[96:128, :], ssinw[64:96, :])
        nc.vector.tensor_mul(t2[96:128, :], ps[64:96, :], ssinw[96:128, :])
    t1 = pool.tile([128, n], DT, tag="rope_t1", name="rope_t1")
    nc.vector.tensor_mul(t1[:], ps[:], cosw[:])
    if not isinstance(out_ap, list):
        out_ap = [out_ap[0:64, :], out_ap[64:128, :]]
    for i, half in enumerate(out_ap):
        add_engine.tensor_add(half[:], t1[64 * i:64 * (i + 1), :],
                              t2[64 * i:64 * (i + 1), :])


def build_program():
    nc = bacc.Bacc("TRN2", target_bir_lowering=False, debug=False,
                   num_devices=NCORE)

    xkv_d = nc.dram_tensor("xkv", [C, KVR], F32R, kind="ExternalInput")
    wq_d = nc.dram_tensor("wq", [C, NH * D], F32R, kind="ExternalInput")
    wk_d = nc.dram_tensor("wk", [C, NKV * D], F32R, kind="ExternalInput")
    wv_d = nc.dram_tensor("wv", [C, NKV * D], F32R, kind="ExternalInput")
    wo_d = nc.dram_tensor("wo", [NH * D, C], F32R, kind="ExternalInput")
    rqc_d = nc.dram_tensor("rope_q_cos", [128, CH], DT, kind="ExternalInput")
    rqs_d = nc.dram_tensor("rope_q_sin", [128, CH], DT, kind="ExternalInput")
    rkc_d = nc.dram_tensor("rope_k_cos", [128, KVR], DT, kind="ExternalInput")
    rks_d = nc.dram_tensor("rope_k_sin", [128, KVR], DT, kind="ExternalInput")
    kvv_d = nc.dram_tensor("kvvalid", [128, NKB], F32R, kind="ExternalInput")
    mw_d = nc.dram_tensor("mask_win8", [128, 512], F32R, kind="ExternalInput")
    mc_d = nc.dram_tensor("mask_causal8", [128, 512], F32R,
                          kind="ExternalInput")
    out_d = nc.dram_tensor("out", [CH, C], DT, kind="ExternalOutput")

    NTH = 3          # x-column thirds
    QW = KVR // NTH  # 512 columns per third

    with tile.TileContext(nc) as tc:
        with (
            tc.tile_pool(name="const", bufs=1) as cpool,
            tc.tile_pool(name="kT", bufs=1) as kT_pool,
            tc.tile_pool(name="vext", bufs=1) as v_pool,
            tc.tile_pool(name="xq_res", bufs=1) as xq_pool,
            tc.tile_pool(name="wq_s", bufs=3) as wq_pool,
            tc.tile_pool(name="rq_tab", bufs=1) as rq_pool,
            tc.tile_pool(name="rope_tmp", bufs=1) as rtmp,
            tc.tile_pool(name="qT0", bufs=1) as qT0_pool,
        ):
            kvv = cpool.tile([128, NKB], F32R, tag="kvv", name="kvv")
            nc.gpsimd.dma_start(kvv[:], kvv_d[:, :])
            mask_win = cpool.tile([128, 512], F32R, tag="mw",
                                  name="mask_win")
            nc.gpsimd.dma_start(mask_win[:], mw_d[:, :])
            mask_causal = cpool.tile([128, 512], F32R, tag="mc",
                                     name="mask_causal")
            nc.gpsimd.dma_start(mask_causal[:], mc_d[:, :])

            warm = cpool.tile([128, 4], DT, tag="warm", name="warm")
            nc.scalar.activation(warm[:], mask_win[0:128, 0:4],
                                 mybir.ActivationFunctionType.Exp)

            # kT: [d, t] packed 2 kv heads per tile.
            kT = [kT_pool.tile([128, KVR], F32R, tag=f"kT{i}", name=f"kT{i}")
                  for i in range(NKV // 2)]
            # vext: one tile, head kvh at pitch VP; per block 64 dims+validity
            vext = v_pool.tile([128, NKV * VP], F32R, tag="vext", name="vext")
            # resident x tiles for the core's own 512 tokens: the third
            # qu==2 of xkv, doubling as the Q-projection moving operand.
            xq = [xq_pool.tile([128, CH], F32R, tag=f"xq{ci}",
                               name=f"xq{ci}") for ci in range(FD)]
            rqc = rq_pool.tile([128, CH], DT, tag="rqc", name="rqc")
            rqs = rq_pool.tile([128, CH], DT, tag="rqs", name="rqs")
            nc.gpsimd.dma_start(rqc[:], rqc_d[:, :])
            nc.gpsimd.dma_start(rqs[:], rqs_d[:, :])
            # qT[p]: rows 0:64 = group 2p (4 heads side by side), rows
            # 64:128 = group 2p+1.  qT[0] has its own buffer; qT[1..3]
            # rotate a 2-buffer pool.
            qT = [None] * 4

            def q_ci_round(sweep, ci, qps, m4s, with_xq_dma=False):
                """One contraction step of a half-sweep of the q proj."""
                if with_xq_dma:
                    nc.gpsimd.dma_start(
                        xq[ci][:],
                        xkv_d[128 * ci:128 * (ci + 1), WIN:WIN + CH])
                m4s = list(m4s)
                wt = wq_pool.tile([128, 256], F32R, tag="wqh",
                                  name="wqh", bufs=4)
                c0 = 512 * sweep + 128 * m4s[0]
                nc.sync.dma_start(
                    wt[:], wq_d[128 * ci:128 * (ci + 1), c0:c0 + 256])
                for j, m4 in enumerate(m4s):
                    nc.tensor.matmul(qps[j][:],
                                     wt[:, 128 * j:128 * (j + 1)],
                                     xq[ci][:], start=(ci == 0),
                                     stop=(ci == FD - 1))

            def q_rope_half(sweep, qps, pool, m4s, swap_engine=None,
                            stage_engine=None):
                for j, m4 in enumerate(m4s):
                    m = 4 * sweep + m4
                    boff = 64 * ((m // 2) % 2)
                    c0 = 512 * (2 * (m % 2))
                    _rope_write(
                        nc, pool,
                        [qT[sweep][boff:boff + 64, c0:c0 + 512],
                         qT[sweep][boff:boff + 64, c0 + 512:c0 + 1024]],
                        qps[j][:], rqc[:], rqs[:], CH,
                        add_engine=nc.gpsimd, swap_engine=swap_engine,
                        stage_engine=stage_engine)

            # ================= KV projection =================
            with (
                tc.tile_pool(name="rk_tab", bufs=1) as rk_pool,
                tc.tile_pool(name="wk_res", bufs=1) as wk_pool,
                tc.tile_pool(name="wv_res", bufs=1) as wv_pool,
                tc.tile_pool(name="xkv_s", bufs=7) as xkv_pool,
                tc.tile_pool(name="ps_kv", bufs=1, space="PSUM") as ps_kv,
            ):
                # wv/wk interleaved so arrivals track the V-lead and the
                # V+K rounds (wv[ci] needed ~1.7us*ci, wk[ci] shortly
                # after).
                wvt = {}
                wkt = {}

                def wv_dma(ci):
                    wvt[ci] = wv_pool.tile([128, NKV * D], F32R,
                                           tag=f"wv{ci}", name=f"wv{ci}")
                    nc.scalar.dma_start(
                        wvt[ci][:], wv_d[128 * ci:128 * (ci + 1), :])

                def wk_dma(ci):
                    wkt[ci] = wk_pool.tile([128, NKV * D], F32R,
                                           tag=f"wk{ci}", name=f"wk{ci}")
                    nc.scalar.dma_start(
                        wkt[ci][:], wk_d[128 * ci:128 * (ci + 1), :])

                for ci in range(3):
                    wv_dma(ci)
                for i in range(FD - 3):
                    wv_dma(i + 3)
                    wk_dma(i)
                for i in range(FD - 3, FD):
                    wk_dma(i)
                rkc = rk_pool.tile([128, KVR], DT, tag="rkc", name="rkc")
                nc.gpsimd.dma_start(rkc[:], rkc_d[:, :])
                rks = rk_pool.tile([128, KVR], DT, tag="rks", name="rks")
                nc.gpsimd.dma_start(rks[:], rks_d[:, :])

                NLEAD = 4     # V-only lead rounds covering the PSUM WAR
                for qu in range(NTH):
                    qs = QW * qu
                    xt = [None] * FD

                    def get_xt(ci):
                        if qu == 2:
                            return xq[ci]
                        if xt[ci] is None:
                            t = xkv_pool.tile([128, QW], F32R, tag="xkv",
                                              name="xkv")
                            nc.sync.dma_start(
                                t[:],
                                xkv_d[128 * ci:128 * (ci + 1), qs:qs + QW])
                            xt[ci] = t
                        return xt[ci]

                    vps = [ps_kv.tile([128, NKV * D], DT, tag=f"vps{st}",
                                      name=f"vps{st}")
                           for st in range(QW // 128)]
                    kps = [ps_kv.tile([128, QW], DT, tag=f"kps{m}",
                                      name=f"kps{m}")
                           for m in range(NKV // 2)]

                    def v_round(ci):
                        x = get_xt(ci)
                        for st in range(QW // 128):
                            nc.tensor.matmul(
                                vps[st][:], x[:, 128 * st:128 * (st + 1)],
                                wvt[ci][:], start=(ci == 0),
                                stop=(ci == FD - 1))

                    def k_round(ci, first, last):
                        x = get_xt(ci)
                        for m in range(NKV // 2):
                            nc.tensor.matmul(
                                kps[m][:],
                                wkt[ci][:, 128 * m:128 * (m + 1)], x[:],
                                start=first, stop=last)

                    def vext_copies():
                        for st in range(QW // 128):
                            tl = (QW // 128) * qu + st   # kv block 0..11
                            nc.scalar.copy(
                                vext[:].rearrange("p (h b w) -> p h b w",
                                                  h=NKV, b=NKB)[:, :, tl,
                                                                0:D],
                                vps[st][:].rearrange("p (h d) -> p h d",
                                                     h=NKV))
                        t0 = (QW // 128) * qu
                        nc.scalar.copy(
                            vext[:].rearrange("p (h b w) -> p h b w",
                                              h=NKV, b=NKB)[
                                                  :, :, t0:t0 + QW // 128,
                                                  D:D + 1],
                            kvv[:, t0:t0 + QW // 128].rearrange(
                                "p (o b) -> p o b", o=1).to_broadcast(
                                    (128, NKV, QW // 128)))

                    # V-only lead (prev third's PSUM WAR drains), then V+K
                    # interleaved, then the deferred K rounds; vext copies
                    # run during the deferred K rounds.
                    for ci in range(NLEAD):
                        v_round(ci)
                    k_round(0, True, False)
                    for ci in range(NLEAD, FD):
                        v_round(ci)
                        k_round(ci, False, False)
                    vext_copies()
                    for ci in range(1, NLEAD):
                        k_round(ci, False, ci == NLEAD - 1)

                    # xq tiles are needed from third 2 on; emitting their
                    # DMAs here keeps the early pipe free for xkv/wv/wk.
                    if qu == 1:
                        for ci in range(FD):
                            nc.gpsimd.dma_start(
                                xq[ci][:],
                                xkv_d[128 * ci:128 * (ci + 1),
                                      WIN:WIN + CH])

                    # ropes: stage ALL kps out of PSUM first (DVE, fast,
                    # frees the banks while ACT does the vext copies),
                    # then rope off the critical path.
                    stgs = []
                    for m in range(NKV // 2):
                        stg = rtmp.tile([128, QW], DT, tag=f"rope_stg{m}",
                                        name=f"rope_stg{m}")
                        nc.vector.tensor_copy(stg[:], kps[m][:])
                        stgs.append(stg)
                    for m in range(NKV // 2):
                        _rope_write(nc, rtmp, kT[m][:, qs:qs + QW],
                                    stgs[m][:], rkc[:, qs:qs + QW],
                                    rks[:, qs:qs + QW], QW,
                                    add_engine=nc.gpsimd,
                                    swap_engine=(nc.scalar if qu == 2
                                                 else nc.gpsimd))

            # ============ q sweep 0 (covers the last kv rope drain) ======
            with tc.tile_pool(name="ps_q0", bufs=1, space="PSUM") as ps_q0:
                qT[0] = qT0_pool.tile([128, REP * CH], F32R, tag="qT0",
                                      name="qT0")
                for hs, m4s in enumerate(((2, 3), (0, 1))):
                    qps = [ps_q0.tile([128, CH], DT, tag=f"q0ps{hs}{j}",
                                      name=f"q0ps{hs}{j}") for j in range(2)]
                    for ci in range(FD):
                        q_ci_round(0, ci, qps, m4s, with_xq_dma=False)
                    if hs == 0:
                        q_rope_half(0, qps, rtmp, m4s)
                    else:
                        stgs = []
                        for j in range(2):
                            stg = rtmp.tile([128, CH], DT,
                                            tag=f"rope_stg{j}",
                                            name=f"rope_stg{j}")
                            nc.vector.tensor_copy(stg[:], qps[j][:])
                            stgs.append(stg)
                        q_rope_half(0, stgs, rtmp, m4s,
                                    swap_engine=nc.gpsimd)

            # ======= attention + q sweeps 1-3 + output projection =======
            with (
                tc.tile_pool(name="aT", bufs=1) as aT_pool,
                tc.tile_pool(name="qTb", bufs=2) as qTb_pool,
                tc.tile_pool(name="pt", bufs=5) as pt_pool,
                tc.tile_pool(name="att_small", bufs=2) as sm_pool,
                tc.tile_pool(name="wo_s", bufs=6) as wo_pool,
                tc.tile_pool(name="ostage", bufs=2) as ostage,
            ):
                # aT: attention output, [d, t], 2 heads per tile.
                aT = [aT_pool.tile([128, CH], F32R, tag=f"aT{i}",
                                   name=f"aT{i}") for i in range(NH // 2)]

                def attention_unit(g, qt, st_pool, st_tag, st_cols,
                                   groups):
                    """One (kv-group, q-tile) unit.  The exp runs over
                    GROUPS of kv blocks (e.g. pairs) to amortize the ACT
                    fixed overhead; ST/PT tiles are sized for the largest
                    group and sliced."""
                    kTt, koff = kT[g // 2], 64 * (g % 2)
                    qv = qT[g // 2][koff:koff + 64, :].rearrange(
                        "p (r t) -> p r t", r=REP)[
                            :, :, 128 * qt:128 * (qt + 1)]
                    OT = ps_OT.tile([65, REP * 128], DT, tag="OT",
                                    name="OT", bufs=2)
                    lk = 0
                    for gs in groups:
                        ST = st_pool.tile([128, st_cols], DT, tag=st_tag,
                                          name=st_tag, bufs=2)
                        for j in range(gs):
                            kb = qt + lk + j
                            nc.tensor.matmul(
                                ST[:, 512 * j:512 * (j + 1)].rearrange(
                                    "p (r t) -> p r t", r=REP),
                                kTt[koff:koff + 64,
                                    128 * kb:128 * (kb + 1)],
                                qv, start=True, stop=True)
                        PT = pt_pool.tile([128, 1536], F32R, tag="PT",
                                          name="PT", bufs=2)
                        nc.scalar.activation(
                            PT[:, :512 * gs], ST[:, :512 * gs],
                            mybir.ActivationFunctionType.Exp)
                        for j in range(gs):
                            lkg = lk + j
                            sl = PT[:, 512 * j:512 * (j + 1)]
                            if lkg == 0:
                                nc.vector.tensor_mul(sl, sl, mask_win[:])
                            elif lkg == NWB - 1:
                                nc.vector.tensor_mul(sl, sl,
                                                     mask_causal[:])
                            kb = qt + lkg
                            nc.tensor.matmul(
                                OT[:],
                                vext[:, VP * g + VW * kb:
                                     VP * g + VW * (kb + 1)],
                                sl, start=(lkg == 0),
                                stop=(lkg == NWB - 1))
                        lk += gs
                    rcp = sm_pool.tile([1, REP * 128], DT, tag="rcp",
                                       name="rcp")
                    nc.vector.reciprocal(rcp[:], OT[64:65, :])
                    rcpb = sm_pool.tile([64, REP * 128], DT, tag="rcpb",
                                        name="rcpb")
                    nc.gpsimd.partition_broadcast(rcpb[:], rcp[:])
                    for r in range(REP):
                        h = REP * g + r
                        nc.vector.tensor_mul(
                            aT[h // 2][64 * (h % 2):64 * (h % 2) + 64,
                                       128 * qt:128 * (qt + 1)],
                            OT[0:64, 128 * r:128 * (r + 1)],
                            rcpb[:, 128 * r:128 * (r + 1)])

                # wo streaming: rolling lookahead of 4 k-rounds across
                # (oc, k); the first tiles are requested at phase-3 start.
                wo_rounds = [(oc, k) for oc in range(4) for k in range(FD)]
                wo_tiles = {}
                wo_next = [0]

                def wo_prefetch(n=1):
                    for _ in range(n):
                        if wo_next[0] >= len(wo_rounds):
                            return
                        oc, k = wo_rounds[wo_next[0]]
                        wo_next[0] += 1
                        wt = wo_pool.tile([128, 512], F32R, tag="wo",
                                          name="wo")
                        nc.gpsimd.dma_start(
                            wt[:], wo_d[128 * k:128 * (k + 1),
                                        512 * oc:512 * (oc + 1)])
                        wo_tiles[(oc, k)] = wt

                def oc_round(k, oc, ops):
                    wot = wo_tiles.pop((oc, k))
                    for tt in range(NQT):
                        nc.tensor.matmul(
                            ops[tt][:], aT[k][:, 128 * tt:128 * (tt + 1)],
                            wot[:], start=(k == 0), stop=(k == FD - 1))
                    wo_prefetch()

                def oc_store(oc, ops):
                    for tt in range(NQT):
                        st = ostage.tile([128, 512], DT, tag="stage",
                                         name="stage")
                        nc.vector.tensor_copy(st[:], ops[tt][:])
                        nc.scalar.dma_start(
                            out_d[128 * tt:128 * (tt + 1),
                                  512 * oc:512 * (oc + 1)], st[:])

                def q_rope_staged(sweep, qps, pool, m4s):
                    """Stage both qps tiles out of PSUM on ACT (frees the
                    banks fast), then swap-rope on Pool."""
                    stgs = []
                    for j in range(len(m4s)):
                        stg = pool.tile([128, CH], DT, tag=f"rope_stg{j}",
                                        name=f"rope_stg{j}")
                        nc.scalar.copy(stg[:], qps[j][:])
                        stgs.append(stg)
                    q_rope_half(sweep, stgs, pool, m4s,
                                swap_engine=nc.gpsimd)

                def emit_phase(p, ps_q, ps_ST):
                    """Attention pair p + q sweep p+1 in two half-sweeps
                    (half B ropes mid-phase; the next phase's first group
                    2p+3 reads only half B's qT rows)."""
                    qT[p + 1] = qTb_pool.tile([128, REP * CH], F32R,
                                              tag="qTb", name="qTb")
                    for hs, (gi, m4s) in enumerate(
                            ((1, (2, 3)), (0, (0, 1)))):
                        g = 2 * p + gi
                        qps = [ps_q.tile([128, CH], DT, tag=f"qps{j}",
                                         name=f"qps{j}")
                               for j in range(2)]
                        ci = 0
                        for qt in range(NQT):
                            attention_unit(g, qt, ps_ST, "STP", 1024,
                                           (2, 2, 2, 2, 1))
                            for _ in range(4):
                                if ci < FD:
                                    q_ci_round(p + 1, ci, qps, m4s)
                                    ci += 1
                        q_rope_staged(p + 1, qps, rtmp, m4s)

                with tc.tile_pool(name="ps_OT", bufs=1,
                                  space="PSUM") as ps_OT:
                    with (
                        tc.tile_pool(name="ps_ST", bufs=1,
                                     space="PSUM") as ps_ST,
                        tc.tile_pool(name="ps_q", bufs=1,
                                     space="PSUM") as ps_q,
                    ):
                        for p in range(3):
                            emit_phase(p, ps_q, ps_ST)
                    # --- phase 3: attention only, triple-batched exps ---
                    with tc.tile_pool(name="ps_ST3", bufs=1,
                                      space="PSUM") as ps_ST3:
                        wo_prefetch(4)
                        for g in (7, 6):
                            for qt in range(NQT):
                                attention_unit(g, qt, ps_ST3, "ST3", 1536,
                                               (3, 3, 3))

                # --- output projection: oc0..3, double-buffered PSUM ---
                with tc.tile_pool(name="ps_o2", bufs=2,
                                  space="PSUM") as ps_o2:
                    for oc in range(4):
                        ops = [ps_o2.tile([128, 512], DT, tag=f"ops2_{tt}",
                                          name=f"ops2_{tt}")
                               for tt in range(NQT)]
                        for k in range(FD):
                            oc_round(k, oc, ops)
                        oc_store(oc, ops)

    nc.compile()
    return nc


def _rope_tables(t_idx, scale):
    """cos + pre-shuffled signed sin tables in [d, t] layout, 2-head
    packed to 128 partitions.

    cos rows 0-63 and 64-127 identical.  The sin table is pre-shuffled:
    w[s] = sign(sig(s)) * sin(ang[sig(s) mod 32]) so that the kernel's
    u = ps*w needs only a block-swap folded into the final add."""
    inv_freq = 1.0 / (ROPE_BASE ** (np.arange(0, D, 2, dtype=np.float64) / D))
    ang = t_idx[None, :] * inv_freq[:, None]          # [32, n]
    cos1 = np.cos(ang)
    sin1 = np.sin(ang)
    cos64 = np.concatenate([cos1, cos1], 0) * scale   # [64, n]
    sin64 = np.concatenate([-sin1, sin1], 0) * scale  # [64, n] signed
    return (np.tile(cos64, (2, 1)).astype(np.float32),
            np.tile(sin64, (2, 1)).astype(np.float32))


def make_in_maps(x, Wq, Wk, Wv, Wo):
    x = np.asarray(x, np.float32)
    ins = []
    i = np.arange(128)
    masks = {
        "mask_win8": np.tile((i[:, None] > i[None, :]).astype(np.float32),
                             (1, REP)),
        "mask_causal8": np.tile((i[:, None] <= i[None, :]).astype(np.float32),
                                (1, REP)),
    }
    for c in range(NCORE):
        b, ch = divmod(c, 4)
        r0 = CH * ch
        kv0 = r0 - WIN
        xT = np.ascontiguousarray(x[b].T)             # [C, T]
        xkv = np.zeros((C, KVR), np.float32)
        pad = max(0, -kv0)
        xkv[:, pad:] = xT[:, kv0 + pad:r0 + CH]
        qc, qs = _rope_tables(np.arange(r0, r0 + CH, dtype=np.float64), SCALE)
        kc, ks = _rope_tables(np.arange(kv0, r0 + CH, dtype=np.float64), 1.0)
        kvvalid = np.zeros((128, NKB), np.float32)
        for lk in range(NKB):
            kvvalid[:, lk] = (kv0 + 128 * lk + i >= 0).astype(np.float32)
        ins.append({
            "xkv": xkv,
            "wq": np.ascontiguousarray(Wq, np.float32),
            "wk": np.ascontiguousarray(Wk, np.float32),
            "wv": np.ascontiguousarray(Wv, np.float32),
            "wo": np.ascontiguousarray(Wo, np.float32),
            "rope_q_cos": qc, "rope_q_sin": qs,
            "rope_k_cos": kc, "rope_k_sin": ks,
            "kvvalid": kvvalid,
            **masks,
        })
    return ins


_PROG_CACHE = {}


def get_program():
    if "nc" not in _PROG_CACHE:
        _PROG_CACHE["nc"] = build_program()
    return _PROG_CACHE["nc"]


def kernel(x, Wq, Wk, Wv, Wo):
    nc = get_program()
    ins = make_in_maps(x, Wq, Wk, Wv, Wo)
    res = run_bass_kernel_spmd(nc, ins, list(range(NCORE)))
    out = np.empty((B, T, C), np.float32)
    for c in range(NCORE):
        b, ch = divmod(c, 4)
        out[b, CH * ch:CH * (ch + 1), :] = res.results[c]["out"]
    return out


# revision 39
# speedup vs baseline: 1.1153x; 1.1153x over previous
"""Trainium2 Bass kernel for Mistral-style sliding-window GQA attention.

Problem: B=2, T=2048, C=2048, 32 q heads / 8 kv heads, head_dim=64,
sliding causal window 1024, RoPE, fp32.

Sharding (sequence-parallel, no cross-core communication):
  core c in 0..7 handles batch b=c//4 and contiguous 512-row chunk k=c%4.
  Each core computes q for its 512 rows, k/v for its rows plus a 1024-row
  halo (zero-padded before t=0), full attention for its rows over all 32
  heads, and the output projection for its rows.  Host gathers by
  concatenation only.

Schedule (v5 — keeps the PE continuously fed):
  - KV projection per 512-col third: V-only rounds for ci 0..3 (covering
    the previous third's PSUM WAR), then V+K interleaved rounds, then the
    deferred K rounds for ci 1..3.  x tiles die right after their K
    round, so the stream pool needs only 8 buffers; wk/wv stay resident.
  - RoPE uses a host-pre-shuffled sin table: out = ps*cos + shuf(ps*sinp)
    where the rotate-half shuffle folds into the four quarter ADDs — no
    shuffle copies at all.  kv ropes stage kps PSUM->SBUF with a fast ACT
    copy (frees the PSUM banks inside the next third's V-lead); muls run
    on DVE, adds on Pool (split DVE/Pool for the last third).
  - q sweep 0 runs right after kv (its PE work covers the last third's
    rope drain); its wq tiles prefetch during kv from a top-level pool.
    Sweeps run as HALF-sweeps: group 2p only needs qT rows 0:64 (m4 0,1),
    so half A's rope runs mid-phase with ~20us of slack.
  - The xkv third covering the core's own 512 tokens doubles as the
    Q-projection moving operand (resident tiles, no re-DMA).
  - Q sweeps 1..3 interleave with attention as per-ci rounds filling the
    PE bubbles left by the ACT-bound exp chain; each half-sweep has its
    own 2 PSUM banks so WAR never stalls the interleave.
  - Output projection: oc0's k-rounds interleave into the last attention
    phase; wo tiles stream with a rolling 6-round lookahead from phase 2;
    oc1..3 run after ps_att closes, on double-buffered PSUM banks.
  - scores in S^T=[key, query] layout with the 4 q heads of a kv group
    packed in the moving operand (N=512, keeps float32r at 1 cyc/row);
    PV uses V_ext (64 dims + validity column) as stationary so row 64
    accumulates the softmax denominator; no max-subtraction (inputs are
    N(0,1)-scaled, exp stays in fp32 range).
"""

import ml_dtypes
import numpy as np

import concourse.bass as bass
import concourse.mybir as mybir
import concourse.tile as tile
from concourse import bacc
from concourse.bass_utils import run_bass_kernel_spmd

B, T, C = 2, 2048, 2048
NH, NKV, D = 32, 8, 64
REP = NH // NKV
WIN = 1024
CH = 512          # q rows per core
KVR = CH + WIN    # kv rows per core (with halo)
NCORE = 8
DT = mybir.dt.float32
F32R = mybir.dt.float32r
SCALE = 1.0 / np.sqrt(np.float32(D))
ROPE_BASE = 10000.0

FD = T // 128     # 16 contraction tiles of the model dim
NQT = CH // 128   # 4 q tiles per chunk
NKB = KVR // 128  # 12 kv blocks per core
NWB = 9           # kv blocks in the window of one q tile
VW = 65           # v_ext width per kv block (64 dims + validity column)
VP = NKB * VW     # per-head v_ext pitch (780)


def _rope_write(nc, pool, out_ap, ps, cosw, ssinw, n, add_engine,
                swap_engine=None, stage_engine=None):
    """out = ps*cos + rot_half(ps)*sin on a [128, n] 2-head-packed tile.

    ssinw rows carry the rotate-half signs (rows 0-31/64-95 negated) and
    any folded scale; cosw carries the same scale.  out_ap is either one
    [128, n] AP or a list of two [64, n] halves.

    Two modes:
      - swap_engine=None ("psum" mode): 4 quarter multiplies read ps
        directly with shifted partition bases (legal because ps is PSUM),
        plus the full-width cos multiply, all on DVE.
      - swap_engine given: ps is optionally staged to SBUF (stage_engine,
        frees the PSUM bank fast), the rotate-half shuffle materializes
        via 4 quarter copies on swap_engine, then 2 full DVE multiplies.
    The final adds run on add_engine.
    """
    if swap_engine is not None:
        if stage_engine is not None:
            stg = pool.tile([128, n], DT, tag="rope_stg", name="rope_stg")
            scp = (stage_engine.copy if hasattr(stage_engine, "copy")
                   else stage_engine.tensor_copy)
            scp(stg[:], ps[:])
            ps = stg
        sw = pool.tile([128, n], DT, tag="rope_sw", name="rope_sw")
        cp = (swap_engine.copy if hasattr(swap_engine, "copy")
              else swap_engine.tensor_copy)
        cp(sw[0:32, :], ps[32:64, :])
        cp(sw[32:64, :], ps[0:32, :])
        cp(sw[64:96, :], ps[96:128, :])
        cp(sw[96:128, :], ps[64:96, :])
        t2 = pool.tile([128, n], DT, tag="rope_t2", name="rope_t2")
        nc.vector.tensor_mul(t2[:], sw[:], ssinw[:])
    else:
        t2 = pool.tile([128, n], DT, tag="rope_t2", name="rope_t2")
        nc.vector.tensor_mul(t2[0:32, :], ps[32:64, :], ssinw[0:32, :])
        nc.vector.tensor_mul(t2[32:64, :], ps[0:32, :], ssinw[32:64, :])
        nc.vector.tensor_mul(t2[64:96, :], ps[96:128, :], ssinw[64:96, :])
        nc.vector.tensor_mul(t2[96:128, :], ps[64:96, :], ssinw[96:128, :])
    t1 = pool.tile([128, n], DT, tag="rope_t1", name="rope_t1")
    nc.vector.tensor_mul(t1[:], ps[:], cosw[:])
    if not isinstance(out_ap, list):
        out_ap = [out_ap[0:64, :], out_ap[64:128, :]]
    for i, half in enumerate(out_ap):
        add_engine.tensor_add(half[:], t1[64 * i:64 * (i + 1), :],
                              t2[64 * i:64 * (i + 1), :])


def build_program():
    nc = bacc.Bacc("TRN2", target_bir_lowering=False, debug=False,
                   num_devices=NCORE)

    xkv_d = nc.dram_tensor("xkv", [C, KVR], F32R, kind="ExternalInput")
    wq_d = nc.dram_tensor("wq", [C, NH * D], F32R, kind="ExternalInput")
    wk_d = nc.dram_tensor("wk", [C, NKV * D], F32R, kind="ExternalInput")
    wv_d = nc.dram_tensor("wv", [C, NKV * D], F32R, kind="ExternalInput")
    wo_d = nc.dram_tensor("wo", [NH * D, C], F32R, kind="ExternalInput")
    rqc_d = nc.dram_tensor("rope_q_cos", [128, CH], DT, kind="ExternalInput")
    rqs_d = nc.dram_tensor("rope_q_sin", [128, CH], DT, kind="ExternalInput")
    rkc_d = nc.dram_tensor("rope_k_cos", [128, KVR], DT, kind="ExternalInput")
    rks_d = nc.dram_tensor("rope_k_sin", [128, KVR], DT, kind="ExternalInput")
    kvv_d = nc.dram_tensor("kvvalid", [128, NKB], F32R, kind="ExternalInput")
    mw_d = nc.dram_tensor("mask_win8", [128, 512], F32R, kind="ExternalInput")
    mc_d = nc.dram_tensor("mask_causal8", [128, 512], F32R,
                          kind="ExternalInput")
    out_d = nc.dram_tensor("out", [CH, C], DT, kind="ExternalOutput")

    NTH = 3          # x-column thirds
    QW = KVR // NTH  # 512 columns per third

    with tile.TileContext(nc) as tc:
        with (
            tc.tile_pool(name="const", bufs=1) as cpool,
            tc.tile_pool(name="kT", bufs=1) as kT_pool,
            tc.tile_pool(name="vext", bufs=1) as v_pool,
            tc.tile_pool(name="xq_res", bufs=1) as xq_pool,
            tc.tile_pool(name="wq_s", bufs=3) as wq_pool,
            tc.tile_pool(name="rq_tab", bufs=1) as rq_pool,
            tc.tile_pool(name="rope_tmp", bufs=1) as rtmp,
            tc.tile_pool(name="qT0", bufs=1) as qT0_pool,
        ):
            kvv = cpool.tile([128, NKB], F32R, tag="kvv", name="kvv")
            nc.gpsimd.dma_start(kvv[:], kvv_d[:, :])
            mask_win = cpool.tile([128, 512], F32R, tag="mw",
                                  name="mask_win")
            nc.gpsimd.dma_start(mask_win[:], mw_d[:, :])
            mask_causal = cpool.tile([128, 512], F32R, tag="mc",
                                     name="mask_causal")
            nc.gpsimd.dma_start(mask_causal[:], mc_d[:, :])

            # kT: [d, t] packed 2 kv heads per tile.
            kT = [kT_pool.tile([128, KVR], F32R, tag=f"kT{i}", name=f"kT{i}")
                  for i in range(NKV // 2)]
            # vext: one tile, head kvh at pitch VP; per block 64 dims+validity
            vext = v_pool.tile([128, NKV * VP], F32R, tag="vext", name="vext")
            # resident x tiles for the core's own 512 tokens: the third
            # qu==2 of xkv, doubling as the Q-projection moving operand.
            xq = [xq_pool.tile([128, CH], F32R, tag=f"xq{ci}",
                               name=f"xq{ci}") for ci in range(FD)]
            rqc = rq_pool.tile([128, CH], DT, tag="rqc", name="rqc")
            rqs = rq_pool.tile([128, CH], DT, tag="rqs", name="rqs")
            nc.gpsimd.dma_start(rqc[:], rqc_d[:, :])
            nc.gpsimd.dma_start(rqs[:], rqs_d[:, :])
            # qT[p]: rows 0:64 = group 2p (4 heads side by side), rows
            # 64:128 = group 2p+1.  qT[0] has its own buffer; qT[1..3]
            # rotate a 2-buffer pool.
            qT = [None] * 4

            def q_ci_round(sweep, ci, qps, m4s, with_xq_dma=False):
                """One contraction step of a half-sweep of the q proj."""
                if with_xq_dma:
                    nc.gpsimd.dma_start(
                        xq[ci][:],
                        xkv_d[128 * ci:128 * (ci + 1), WIN:WIN + CH])
                m4s = list(m4s)
                wt = wq_pool.tile([128, 256], F32R, tag="wqh",
                                  name="wqh", bufs=4)
                c0 = 512 * sweep + 128 * m4s[0]
                nc.sync.dma_start(
                    wt[:], wq_d[128 * ci:128 * (ci + 1), c0:c0 + 256])
                for j, m4 in enumerate(m4s):
                    nc.tensor.matmul(qps[j][:],
                                     wt[:, 128 * j:128 * (j + 1)],
                                     xq[ci][:], start=(ci == 0),
                                     stop=(ci == FD - 1))

            def q_rope_half(sweep, qps, pool, m4s, swap_engine=None,
                            stage_engine=None):
                for j, m4 in enumerate(m4s):
                    m = 4 * sweep + m4
                    boff = 64 * ((m // 2) % 2)
                    c0 = 512 * (2 * (m % 2))
                    _rope_write(
                        nc, pool,
                        [qT[sweep][boff:boff + 64, c0:c0 + 512],
                         qT[sweep][boff:boff + 64, c0 + 512:c0 + 1024]],
                        qps[j][:], rqc[:], rqs[:], CH,
                        add_engine=nc.gpsimd, swap_engine=swap_engine,
                        stage_engine=stage_engine)

            # ================= KV projection =================
            with (
                tc.tile_pool(name="rk_tab", bufs=1) as rk_pool,
                tc.tile_pool(name="wk_res", bufs=1) as wk_pool,
                tc.tile_pool(name="wv_res", bufs=1) as wv_pool,
                tc.tile_pool(name="xkv_s", bufs=7) as xkv_pool,
                tc.tile_pool(name="ps_kv", bufs=1, space="PSUM") as ps_kv,
            ):
                # wv/wk interleaved so arrivals track the V-lead and the
                # V+K rounds (wv[ci] needed ~1.7us*ci, wk[ci] shortly
                # after).
                wvt = {}
                wkt = {}

                def wv_dma(ci):
                    wvt[ci] = wv_pool.tile([128, NKV * D], F32R,
                                           tag=f"wv{ci}", name=f"wv{ci}")
                    nc.scalar.dma_start(
                        wvt[ci][:], wv_d[128 * ci:128 * (ci + 1), :])

                def wk_dma(ci):
                    wkt[ci] = wk_pool.tile([128, NKV * D], F32R,
                                           tag=f"wk{ci}", name=f"wk{ci}")
                    nc.scalar.dma_start(
                        wkt[ci][:], wk_d[128 * ci:128 * (ci + 1), :])

                for ci in range(3):
                    wv_dma(ci)
                for i in range(FD - 3):
                    wv_dma(i + 3)
                    wk_dma(i)
                for i in range(FD - 3, FD):
                    wk_dma(i)
                rkc = rk_pool.tile([128, KVR], DT, tag="rkc", name="rkc")
                nc.gpsimd.dma_start(rkc[:], rkc_d[:, :])
                rks = rk_pool.tile([128, KVR], DT, tag="rks", name="rks")
                nc.gpsimd.dma_start(rks[:], rks_d[:, :])

                NLEAD = 4     # V-only lead rounds covering the PSUM WAR
                for qu in range(NTH):
                    qs = QW * qu
                    xt = [None] * FD

                    def get_xt(ci):
                        if qu == 2:
                            return xq[ci]
                        if xt[ci] is None:
                            t = xkv_pool.tile([128, QW], F32R, tag="xkv",
                                              name="xkv")
                            nc.sync.dma_start(
                                t[:],
                                xkv_d[128 * ci:128 * (ci + 1), qs:qs + QW])
                            xt[ci] = t
                        return xt[ci]

                    vps = [ps_kv.tile([128, NKV * D], DT, tag=f"vps{st}",
                                      name=f"vps{st}")
                           for st in range(QW // 128)]
                    kps = [ps_kv.tile([128, QW], DT, tag=f"kps{m}",
                                      name=f"kps{m}")
                           for m in range(NKV // 2)]

                    def v_round(ci):
                        x = get_xt(ci)
                        for st in range(QW // 128):
                            nc.tensor.matmul(
                                vps[st][:], x[:, 128 * st:128 * (st + 1)],
                                wvt[ci][:], start=(ci == 0),
                                stop=(ci == FD - 1))

                    def k_round(ci, first, last):
                        x = get_xt(ci)
                        for m in range(NKV // 2):
                            nc.tensor.matmul(
                                kps[m][:],
                                wkt[ci][:, 128 * m:128 * (m + 1)], x[:],
                                start=first, stop=last)

                    def vext_copies():
                        for st in range(QW // 128):
                            tl = (QW // 128) * qu + st   # kv block 0..11
                            nc.scalar.copy(
                                vext[:].rearrange("p (h b w) -> p h b w",
                                                  h=NKV, b=NKB)[:, :, tl,
                                                                0:D],
                                vps[st][:].rearrange("p (h d) -> p h d",
                                                     h=NKV))
                        t0 = (QW // 128) * qu
                        nc.scalar.copy(
                            vext[:].rearrange("p (h b w) -> p h b w",
                                              h=NKV, b=NKB)[
                                                  :, :, t0:t0 + QW // 128,
                                                  D:D + 1],
                            kvv[:, t0:t0 + QW // 128].rearrange(
                                "p (o b) -> p o b", o=1).to_broadcast(
                                    (128, NKV, QW // 128)))

                    # V-only lead (prev third's PSUM WAR drains), then V+K
                    # interleaved, then the deferred K rounds; vext copies
                    # run during the deferred K rounds.
                    for ci in range(NLEAD):
                        v_round(ci)
                    k_round(0, True, False)
                    for ci in range(NLEAD, FD):
                        v_round(ci)
                        k_round(ci, False, False)
                    vext_copies()
                    for ci in range(1, NLEAD):
                        k_round(ci, False, ci == NLEAD - 1)

                    # xq tiles are needed from third 2 on; emitting their
                    # DMAs here keeps the early pipe free for xkv/wv/wk.
                    if qu == 1:
                        for ci in range(FD):
                            nc.gpsimd.dma_start(
                                xq[ci][:],
                                xkv_d[128 * ci:128 * (ci + 1),
                                      WIN:WIN + CH])

                    # ropes: stage ALL kps out of PSUM first (DVE, fast,
                    # frees the banks while ACT does the vext copies),
                    # then rope off the critical path.
                    stgs = []
                    for m in range(NKV // 2):
                        stg = rtmp.tile([128, QW], DT, tag=f"rope_stg{m}",
                                        name=f"rope_stg{m}")
                        nc.vector.tensor_copy(stg[:], kps[m][:])
                        stgs.append(stg)
                    for m in range(NKV // 2):
                        _rope_write(nc, rtmp, kT[m][:, qs:qs + QW],
                                    stgs[m][:], rkc[:, qs:qs + QW],
                                    rks[:, qs:qs + QW], QW,
                                    add_engine=nc.gpsimd,
                                    swap_engine=(nc.scalar if qu == 2
                                                 else nc.gpsimd))

            # ============ q sweep 0 (covers the last kv rope drain) ======
            with tc.tile_pool(name="ps_q0", bufs=1, space="PSUM") as ps_q0:
                qT[0] = qT0_pool.tile([128, REP * CH], F32R, tag="qT0",
                                      name="qT0")
                for hs, m4s in enumerate(((2, 3), (0, 1))):
                    qps = [ps_q0.tile([128, CH], DT, tag=f"q0ps{hs}{j}",
                                      name=f"q0ps{hs}{j}") for j in range(2)]
                    for ci in range(FD):
                        q_ci_round(0, ci, qps, m4s, with_xq_dma=False)
                    if hs == 0:
                        q_rope_half(0, qps, rtmp, m4s)
                    else:
                        stgs = []
                        for j in range(2):
                            stg = rtmp.tile([128, CH], DT,
                                            tag=f"rope_stg{j}",
                                            name=f"rope_stg{j}")
                            nc.vector.tensor_copy(stg[:], qps[j][:])
                            stgs.append(stg)
                        q_rope_half(0, stgs, rtmp, m4s,
                                    swap_engine=nc.gpsimd)

            # ======= attention + q sweeps 1-3 + output projection =======
            with (
                tc.tile_pool(name="aT", bufs=1) as aT_pool,
                tc.tile_pool(name="qTb", bufs=2) as qTb_pool,
                tc.tile_pool(name="rope_tmp_q", bufs=2) as rtmpq,
                tc.tile_pool(name="pt", bufs=5) as pt_pool,
                tc.tile_pool(name="att_small", bufs=2) as sm_pool,
                tc.tile_pool(name="wo_s", bufs=6) as wo_pool,
                tc.tile_pool(name="ostage", bufs=2) as ostage,
            ):
                # aT: attention output, [d, t], 2 heads per tile.
                aT = [aT_pool.tile([128, CH], F32R, tag=f"aT{i}",
                                   name=f"aT{i}") for i in range(NH // 2)]

                def attention_unit(g, qt, ps_a):
                    """One (kv-group, q-tile) unit: 18 matmuls + exp,
                    software-pipelined depth 2 so each PV consumes an exp
                    issued two QK steps earlier."""
                    kTt, koff = kT[g // 2], 64 * (g % 2)
                    qv = qT[g // 2][koff:koff + 64, :].rearrange(
                        "p (r t) -> p r t", r=REP)[
                            :, :, 128 * qt:128 * (qt + 1)]
                    OT = ps_a.tile([65, REP * 128], DT, tag="OT",
                                   name="OT", bufs=2)
                    pts = {}

                    def qk_step(lk):
                        kb = qt + lk
                        ST = ps_a.tile([128, REP * 128], DT, tag="ST",
                                       name="ST", bufs=2)
                        nc.tensor.matmul(
                            ST.rearrange("p (r t) -> p r t", r=REP),
                            kTt[koff:koff + 64, 128 * kb:128 * (kb + 1)],
                            qv, start=True, stop=True)
                        PT = pt_pool.tile([128, REP * 128], F32R,
                                          tag="PT", name="PT", bufs=4)
                        nc.scalar.activation(
                            PT[:], ST[:], mybir.ActivationFunctionType.Exp)
                        if lk == 0:
                            nc.vector.tensor_mul(PT[:], PT[:], mask_win[:])
                        elif lk == NWB - 1:
                            nc.vector.tensor_mul(PT[:], PT[:],
                                                 mask_causal[:])
                        pts[lk] = PT

                    def pv_step(lk):
                        kb = qt + lk
                        nc.tensor.matmul(
                            OT[:],
                            vext[:, VP * g + VW * kb:VP * g + VW * (kb + 1)],
                            pts.pop(lk)[:], start=(lk == 0),
                            stop=(lk == NWB - 1))

                    qk_step(0)
                    qk_step(1)
                    for lk in range(2, NWB):
                        qk_step(lk)
                        pv_step(lk - 2)
                    pv_step(NWB - 2)
                    pv_step(NWB - 1)
                    rcp = sm_pool.tile([1, REP * 128], DT, tag="rcp",
                                       name="rcp")
                    nc.vector.reciprocal(rcp[:], OT[64:65, :])
                    rcpb = sm_pool.tile([64, REP * 128], DT, tag="rcpb",
                                        name="rcpb")
                    nc.gpsimd.partition_broadcast(rcpb[:], rcp[:])
                    for r in range(REP):
                        h = REP * g + r
                        nc.vector.tensor_mul(
                            aT[h // 2][64 * (h % 2):64 * (h % 2) + 64,
                                       128 * qt:128 * (qt + 1)],
                            OT[0:64, 128 * r:128 * (r + 1)],
                            rcpb[:, 128 * r:128 * (r + 1)])

                # wo streaming: rolling lookahead of 6 k-rounds across
                # (oc, k); the first tiles are requested during phase 2.
                oc0_order = [0, 1, 2, 3, 4, 5, 6, 7, 8, 9, 14, 15,
                             10, 11, 12, 13]
                wo_rounds = ([(0, k) for k in oc0_order] +
                             [(oc, k) for oc in range(1, 4)
                              for k in range(FD)])
                wo_tiles = {}
                wo_next = [0]

                def wo_prefetch(n=1):
                    for _ in range(n):
                        if wo_next[0] >= len(wo_rounds):
                            return
                        oc, k = wo_rounds[wo_next[0]]
                        wo_next[0] += 1
                        wt = wo_pool.tile([128, 512], F32R, tag="wo",
                                          name="wo")
                        nc.gpsimd.dma_start(
                            wt[:], wo_d[128 * k:128 * (k + 1),
                                        512 * oc:512 * (oc + 1)])
                        wo_tiles[(oc, k)] = wt

                def oc_round(k, oc, ops):
                    wot = wo_tiles.pop((oc, k))
                    for tt in range(NQT):
                        nc.tensor.matmul(
                            ops[tt][:], aT[k][:, 128 * tt:128 * (tt + 1)],
                            wot[:], start=(k == 0), stop=(k == FD - 1))
                    wo_prefetch()

                def oc_store(oc, ops, chunks=1):
                    for tt in range(NQT):
                        nch = chunks if tt == NQT - 1 else 1
                        w = 512 // nch
                        for cc in range(nch):
                            st = ostage.tile([128, w], DT,
                                             tag=f"stage{nch}{cc}",
                                             name="stage")
                            nc.vector.tensor_copy(
                                st[:], ops[tt][:, w * cc:w * (cc + 1)])
                            nc.scalar.dma_start(
                                out_d[128 * tt:128 * (tt + 1),
                                      512 * oc + w * cc:
                                      512 * oc + w * (cc + 1)], st[:])

                def emit_phase(p, ps_q, ps_a):
                    """Attention pair p + q sweep p+1 in two half-sweeps
                    (half A ropes mid-phase; group 2p+2 reads only half
                    A's qT rows at phase start)."""
                    if p == 2:
                        wo_prefetch(8)
                    qT[p + 1] = qTb_pool.tile([128, REP * CH], F32R,
                                              tag="qTb", name="qTb")
                    for hs, (gi, m4s) in enumerate(
                            ((1, (2, 3)), (0, (0, 1)))):
                        g = 2 * p + gi
                        qps = [ps_q.tile([128, CH], DT, tag=f"qps{hs}{j}",
                                         name=f"qps{hs}{j}")
                               for j in range(2)]
                        ci = 0
                        for qt in range(NQT):
                            attention_unit(g, qt, ps_a)
                            for _ in range(4):
                                if ci < FD:
                                    q_ci_round(p + 1, ci, qps, m4s)
                                    ci += 1
                        q_rope_half(p + 1, qps, rtmpq, m4s)

                def emit_phase3(ps_o, ps_a):
                    """Attention pair 3 + oc0 interleaved."""
                    ops0 = [ps_o.tile([128, 512], DT, tag=f"ops{tt}",
                                      name=f"ops{tt}") for tt in range(NQT)]
                    # aT[k] is complete after unit (k//2, 3); k rounds
                    # 0..11 go behind groups 6/7's units, 12..13 after
                    # group 6 finishes.
                    sched = {(7, 0): [0, 1], (7, 1): [2, 3],
                             (7, 2): [4, 5], (7, 3): [6, 7],
                             (6, 0): [8, 9], (6, 1): [14, 15],
                             (6, 2): [10, 11]}
                    for g in (7, 6):
                        for qt in range(NQT):
                            attention_unit(g, qt, ps_a)
                            for k in sched.get((g, qt), []):
                                oc_round(k, 0, ops0)
                    oc_round(12, 0, ops0)
                    oc_round(13, 0, ops0)
                    oc_store(0, ops0)

                with tc.tile_pool(name="ps_att", bufs=1,
                                  space="PSUM") as ps_att:
                    with tc.tile_pool(name="ps_q", bufs=1,
                                      space="PSUM") as ps_q:
                        for p in range(3):
                            emit_phase(p, ps_q, ps_att)
                    with tc.tile_pool(name="ps_o", bufs=1,
                                      space="PSUM") as ps_o:
                        emit_phase3(ps_o, ps_att)

                # --- oc1..3: double-buffered PSUM (ps_att closed) ---
                with tc.tile_pool(name="ps_o2", bufs=2,
                                  space="PSUM") as ps_o2:
                    for oc in range(1, 4):
                        ops = [ps_o2.tile([128, 512], DT, tag=f"ops2_{tt}",
                                          name=f"ops2_{tt}")
                               for tt in range(NQT)]
                        for k in range(FD):
                            oc_round(k, oc, ops)
                        oc_store(oc, ops, chunks=4 if oc == 3 else 1)

    nc.compile()
    return nc


def _rope_tables(t_idx, scale):
    """cos + pre-shuffled signed sin tables in [d, t] layout, 2-head
    packed to 128 partitions.

    cos rows 0-63 and 64-127 identical.  The sin table is pre-shuffled:
    w[s] = sign(sig(s)) * sin(ang[sig(s) mod 32]) so that the kernel's
    u = ps*w needs only a block-swap folded into the final add."""
    inv_freq = 1.0 / (ROPE_BASE ** (np.arange(0, D, 2, dtype=np.float64) / D))
    ang = t_idx[None, :] * inv_freq[:, None]          # [32, n]
    cos1 = np.cos(ang)
    sin1 = np.sin(ang)
    cos64 = np.concatenate([cos1, cos1], 0) * scale   # [64, n]
    sin64 = np.concatenate([-sin1, sin1], 0) * scale  # [64, n] signed
    return (np.tile(cos64, (2, 1)).astype(np.float32),
            np.tile(sin64, (2, 1)).astype(np.float32))


def make_in_maps(x, Wq, Wk, Wv, Wo):
    x = np.asarray(x, np.float32)
    ins = []
    i = np.arange(128)
    masks = {
        "mask_win8": np.tile((i[:, None] > i[None, :]).astype(np.float32),
                             (1, REP)),
        "mask_causal8": np.tile((i[:, None] <= i[None, :]).astype(np.float32),
                                (1, REP)),
    }
    for c in range(NCORE):
        b, ch = divmod(c, 4)
        r0 = CH * ch
        kv0 = r0 - WIN
        xT = np.ascontiguousarray(x[b].T)             # [C, T]
        xkv = np.zeros((C, KVR), np.float32)
        pad = max(0, -kv0)
        xkv[:, pad:] = xT[:, kv0 + pad:r0 + CH]
        qc, qs = _rope_tables(np.arange(r0, r0 + CH, dtype=np.float64), SCALE)
        kc, ks = _rope_tables(np.arange(kv0, r0 + CH, dtype=np.float64), 1.0)
        kvvalid = np.zeros((128, NKB), np.float32)
        for lk in range(NKB):
            kvvalid[:, lk] = (kv0 + 128 * lk + i >= 0).astype(np.float32)
        ins.append({
            "xkv": xkv,
            "wq": np.ascontiguousarray(Wq, np.float32),
            "wk": np.ascontiguousarray(Wk, np.float32),
            "wv": np.ascontiguousarray(Wv, np.float32),
            "wo": np.ascontiguousarray(Wo, np.float32),
            "rope_q_cos": qc, "rope_q_sin": qs,
            "rope_k_cos": kc, "rope_k_sin": ks,
            "kvvalid": kvvalid,
            **masks,
        })
    return ins


_PROG_CACHE = {}


def get_program():
    if "nc" not in _PROG_CACHE:
        _PROG_CACHE["nc"] = build_program()
    return _PROG_CACHE["nc"]


def kernel(x, Wq, Wk, Wv, Wo):
    nc = get_program()
    ins = make_in_maps(x, Wq, Wk, Wv, Wo)
    res = run_bass_kernel_spmd(nc, ins, list(range(NCORE)))
    out = np.empty((B, T, C), np.float32)
    for c in range(NCORE):
        b, ch = divmod(c, 4)
        out[b, CH * ch:CH * (ch + 1), :] = res.results[c]["out"]
    return out
